# revision 11
# baseline (speedup 1.0000x reference)
"""Trainium2 Bass kernel for EpsilonNetGM (forward-diffused GMM score network).

Math (per row x of shape [D]):
    m'_k   = sqrt(acp) * means_k
    logit_k = (x . m'_k)/sigma2 + [log w_k - 0.5*||m'_k||^2/sigma2]
    resp    = softmax_k(logit)
    out     = c * (x - resp @ m'),   c = 1/sqrt(sigma2),  sigma2 = 1 - acp

Data-parallel over 8 NeuronCores: x/out sharded on the batch axis,
small constants replicated.  Per core (4096 rows), rows are processed in
super-blocks of 512 = 4 sub-blocks of 128 partitions:

  DMA x -> [PE transpose x] -> [PE matmul S^T = (M'/s2)^T xT] ->
  [ACT exp(S^T + logw) = E^T] -> [PE transpose E^T back to row-major] ->
  [DVE segmented row-sums + reciprocal + normalize -> resp (padded K 25->32)] ->
  [PE transpose resp -> resp^T] -> [PE matmul V = resp @ (-c*M')] ->
  [DVE out = c*x + V] -> DMA out.

The exp skips max-subtraction: |logits| <= ~60 here, well inside fp32
exp range, and softmax is shift-invariant so the result matches the
reference to fp32 rounding.
"""

import os
import sys

for _p in ("/opt/trn_rl_repo", "/root/.axon_site/_ro/trn_rl_repo"):
    if os.path.isdir(_p) and _p not in sys.path:
        sys.path.insert(0, _p)

import numpy as np
from contextlib import ExitStack

import concourse.bass as bass
import concourse.bacc as bacc
import concourse.tile as tile
from concourse import mybir
from concourse.bass_utils import run_bass_kernel_spmd

N_CORES = 8
N, K, D = 32768, 25, 128
N_PER = N // N_CORES          # 4096 rows per core
SB = 512                      # rows per super-block
NSB = N_PER // SB             # 8 super-blocks per core
KP = 32                       # K padded to 32 for matmul row-group alignment

F32 = mybir.dt.float32
AX = mybir.AxisListType
OP = mybir.AluOpType
AF = mybir.ActivationFunctionType


def _bcast_free(ap, count):
    """Append a stride-0 free dim of `count` to an AP (broadcast along free)."""
    return bass.AP(ap.tensor, ap.offset, list(ap.ap) + [[0, count]])


def build_program(c_scale: float):
    nc = bacc.Bacc("TRN2", debug=False)

    x_d = nc.dram_tensor("x", [N_PER, D], F32, kind="ExternalInput").ap()
    mts_d = nc.dram_tensor("mts", [D, K], F32, kind="ExternalInput").ap()
    lw_d = nc.dram_tensor("lw", [K, 1], F32, kind="ExternalInput").ap()
    # block-diagonal [-c*M'] replicated per 32-row k-group: one C=128 matmul
    # computes all four 128-row sub-blocks without row-group tile_position
    # (multi-row-group tile_position matmuls fail on HW).
    nm_d = nc.dram_tensor("nm", [128, 4 * D], F32, kind="ExternalInput").ap()
    id_d = nc.dram_tensor("idm", [128, 128], F32, kind="ExternalInput").ap()
    out_d = nc.dram_tensor("out", [N_PER, D], F32, kind="ExternalOutput").ap()

    with tile.TileContext(nc) as tc, ExitStack() as ctx:
        consts = ctx.enter_context(tc.tile_pool(name="consts", bufs=1))
        xin_p = ctx.enter_context(tc.tile_pool(name="xin", bufs=3))
        xt_p = ctx.enter_context(tc.tile_pool(name="xt", bufs=2))
        et_p = ctx.enter_context(tc.tile_pool(name="et", bufs=2))
        small_p = ctx.enter_context(tc.tile_pool(name="small", bufs=3))
        r_p = ctx.enter_context(tc.tile_pool(name="r", bufs=2))
        rt_p = ctx.enter_context(tc.tile_pool(name="rt", bufs=2))
        out_p = ctx.enter_context(tc.tile_pool(name="outp", bufs=3))
        ps_xt = ctx.enter_context(tc.tile_pool(name="ps_xt", bufs=2, space="PSUM"))
        ps_st = ctx.enter_context(tc.tile_pool(name="ps_st", bufs=1, space="PSUM"))
        ps_e = ctx.enter_context(tc.tile_pool(name="ps_e", bufs=1, space="PSUM"))
        ps_rt = ctx.enter_context(tc.tile_pool(name="ps_rt", bufs=1, space="PSUM"))
        ps_v = ctx.enter_context(tc.tile_pool(name="ps_v", bufs=2, space="PSUM"))

        mts = consts.tile([D, K], F32)
        nc.sync.dma_start(mts, mts_d)
        lw = consts.tile([K, 1], F32)
        nc.sync.dma_start(lw, lw_d)
        nm = consts.tile([128, 4 * D], F32)
        nc.sync.dma_start(nm, nm_d)
        idm = consts.tile([128, 128], F32)
        nc.sync.dma_start(idm, id_d)

        for s in range(NSB):
            n0 = s * SB
            x_view = x_d[n0:n0 + SB, :].rearrange("(b p) d -> p b d", p=128)
            o_view = out_d[n0:n0 + SB, :].rearrange("(b p) d -> p b d", p=128)

            xin = xin_p.tile([128, SB], F32)
            nc.sync.dma_start(xin.rearrange("p (b d) -> p b d", d=D), x_view)

            # x^T per 128-row sub-block: pxt[d, 128b+p] = x[n0+128b+p, d]
            pxt = ps_xt.tile([128, SB], F32)
            for b in range(4):
                nc.tensor.transpose(
                    pxt[:, 128 * b:128 * (b + 1)],
                    xin[:, 128 * b:128 * (b + 1)],
                    idm,
                )
            xt = xt_p.tile([128, SB], F32)
            nc.vector.tensor_copy(xt, pxt)

            # S^T[k, f] = x_f . m'_k / sigma2
            pst = ps_st.tile([K, SB], F32)
            nc.tensor.matmul(pst, lhsT=mts, rhs=xt, start=True, stop=True)

            # E^T = exp(S^T + logw_adj)   (bias is per-partition = per-k)
            et = et_p.tile([K, SB], F32)
            nc.scalar.activation(et, pst, AF.Exp, bias=lw[:, 0:1], scale=1.0)

            # back to row-major: pe[p, 25b+k] = E[row(b,p), k]
            pe = ps_e.tile([128, 4 * K], F32)
            for b in range(4):
                nc.tensor.transpose(
                    pe[:, K * b:K * (b + 1)],
                    et[:, 128 * b:128 * (b + 1)],
                    idm[0:K, 0:K],
                )

            # segmented softmax denominators
            sum4 = small_p.tile([128, 4], F32)
            nc.vector.reduce_sum(
                sum4, pe.rearrange("p (b k) -> p b k", k=K), axis=AX.X
            )
            recip4 = small_p.tile([128, 4], F32)
            nc.vector.reciprocal(recip4, sum4)

            # normalized responsibilities, K padded to 32 with zeros
            rpad = r_p.tile([128, 4 * KP], F32)
            nc.gpsimd.memset(rpad, 0.0)
            rpad_v = rpad.rearrange("p (b k) -> p b k", k=KP)[:, :, 0:K]
            nc.vector.tensor_mul(
                rpad_v,
                pe.rearrange("p (b k) -> p b k", k=K),
                _bcast_free(recip4, K),
            )

            # resp^T: prt[32b+k, p] = resp[row(b,p), k]
            prt = ps_rt.tile([128, 128], F32)
            nc.tensor.transpose(prt, rpad, idm)
            rt = rt_p.tile([128, 128], F32)
            nc.vector.tensor_copy(rt, prt)

            # V = resp @ (-c*M'): pv[p, 128b+d], via block-diagonal rhs
            pv = ps_v.tile([128, SB], F32)
            nc.tensor.matmul(pv, lhsT=rt, rhs=nm, start=True, stop=True)

            # out = c*x + V
            o4 = out_p.tile([128, SB], F32)
            nc.vector.scalar_tensor_tensor(
                o4, in0=xin, scalar=float(c_scale), in1=pv,
                op0=OP.mult, op1=OP.add,
            )
            nc.sync.dma_start(o_view, o4.rearrange("p (b d) -> p b d", d=D))

    nc.compile()
    return nc


def _host_constants(means, weights, alphas_cumprod, t):
    acp = float(np.asarray(alphas_cumprod, dtype=np.float64)[int(t)])
    sigma2 = 1.0 - acp
    c = 1.0 / np.sqrt(sigma2)
    mprime = np.sqrt(acp) * np.asarray(means, dtype=np.float64)      # [K, D]
    mts = (mprime / sigma2).T.astype(np.float32).copy()              # [D, K]
    logw = np.log(np.asarray(weights, dtype=np.float64))
    lw = (logw - 0.5 * np.sum(mprime * mprime, axis=1) / sigma2)
    lw = lw.astype(np.float32).reshape(K, 1).copy()
    nm = np.zeros((128, 4 * D), dtype=np.float32)
    for b in range(4):
        nm[KP * b:KP * b + K, D * b:D * (b + 1)] = (-c * mprime).astype(np.float32)
    idm = np.eye(128, dtype=np.float32)
    return float(c), mts, lw, nm, idm


def kernel(x, means, weights, alphas_cumprod, t):
    x = np.ascontiguousarray(np.asarray(x, dtype=np.float32))
    assert x.shape == (N, D), x.shape
    c, mts, lw, nm, idm = _host_constants(means, weights, alphas_cumprod, t)

    nc = build_program(c)
    in_maps = []
    for i in range(N_CORES):
        shard = np.ascontiguousarray(x[i * N_PER:(i + 1) * N_PER])
        in_maps.append({"x": shard, "mts": mts, "lw": lw, "nm": nm, "idm": idm})

    res = run_bass_kernel_spmd(nc, in_maps, list(range(N_CORES)))
    out = np.concatenate([res.results[i]["out"] for i in range(N_CORES)], axis=0)
    return out.astype(np.float32, copy=False)


if __name__ == "__main__":
    # smoke test with random data
    rng = np.random.default_rng(0)
    x = rng.standard_normal((N, D), dtype=np.float32)
    means = 2.0 * rng.standard_normal((K, D)).astype(np.float32)
    w = rng.uniform(0.1, 1.0, K).astype(np.float32)
    weights = w / w.sum()
    betas = np.linspace(1e-4, 0.02, 1000, dtype=np.float32)
    acp = np.cumprod(1.0 - betas).astype(np.float32)
    out = kernel(x, means, weights, acp, 500)
    print("out", out.shape, out.dtype, out[:2, :4])
